# revision 1
# baseline (speedup 1.0000x reference)
"""Trainium2 Bass kernel for batched nearest-neighbor min-distance.

Problem: for each row u of U_z [16384, 256], compute
    min_{l in L_z [8192, 256]} ||u - l||_2
Strategy (8 NeuronCores, data-parallel over U rows of U_z, L_z replicated;
`pred` is unused by the reference and ignored):
  d2(u,l) = ||u||^2 + ||l||^2 - 2 u.l
  Per core (2048 U rows, all transposed + pre-scaled on host):
    - SBUF holds L^T [256, 8192] and (-2 U)^T [256, 2048] (fp16).
    - Loop over 64 L-tiles (128 L rows each):
        PSUM[128 Lrows, 2048 Ucols] = (-2 U L^T)^T via 8 matmuls
        (K = 2 x 128 accumulated, moving free dim 512 = one PSUM bank).
        Consumers fold in ||l||^2 - C (per-partition ACT bias) and keep a
        running elementwise min:
          ACT: conv = fp16(psum + l2c)   [the only engine pass over psum]
          DVE: rmin = min(rmin, conv)    [fp16 2x tensor_tensor]
    - Partition reduction via DVE 32x32 block transpose + blocked free-dim
      min + two DMA-realigned tree levels, then add ||u||^2 + C, clamp at
      0, sqrt, DMA out [32, 64] fp32 (column c = 32b + i at [i, b]).
The C=256 shift centers l2 so fp16 intermediates keep ~1e-4 relative error.
Measured (rounds-loop slope, max over 8 cores): ~135 us/exec vs a ~119 us
matmul+DMA floor and a 109 us warm-PE ideal.
"""

import numpy as np

N, M, D = 16384, 8192, 256
CORES = 8
C_SHIFT = 256.0

_COMPILED = {}


def _build(ucols: int, m: int, pattern=None, debug: bool = False, rounds: int = 1,
           fp16_inputs: bool = False, warmup_mms: int = 0,
           conv_bufs: int = 8, bf16_inter: bool = False, psum_width: int = 0,
           act_split: int = 1, dma_split: bool = False):
    """Build + compile the per-core Bass kernel.

    ucols:  number of U columns (rows of U_z) this core handles.
    m:      number of L rows (library size).
    rounds: repeat the whole computation this many times inside a hardware
            loop (benchmarking only -- slope between round counts isolates
            steady-state HW time from the host dispatch overhead).
    """
    from contextlib import ExitStack

    import concourse.bacc as bacc
    import concourse.tile as tile
    from concourse import mybir

    F32 = mybir.dt.float32
    F16 = mybir.dt.bfloat16 if bf16_inter else mybir.dt.float16
    F32R = mybir.dt.float32r
    AF = mybir.ActivationFunctionType
    ALU = mybir.AluOpType

    ltiles = m // 128
    assert ucols % 512 == 0 and m % 128 == 0

    # Measured on HW (rounds-loop slope, per 64-L-tile round): matmul+DMA
    # floor 119 us; ACT full-width conversion ~2.0us/tile paces just above
    # the 1.86us/tile PE period; DVE PSUM reads are 1x (2.7us/tile) so ACT
    # does all conversion and DVE only the fp16 2x running-min merge.
    # act_split>1 and psum_width<ucols both measured slower (per-op fixed
    # costs dominate); conv_bufs=8 >> 4 (buffer starvation throttles ACT).
    in_dt = F16 if fp16_inputs else F32R

    nc = bacc.Bacc("TRN2", target_bir_lowering=False, debug=debug)

    blocks = ucols // 32
    ut_d = nc.dram_tensor("ut", [2, 128, ucols], in_dt, kind="ExternalInput").ap()
    lt_d = nc.dram_tensor("lt", [2, 128, m], in_dt, kind="ExternalInput").ap()
    l2c_d = nc.dram_tensor("l2c", [128, ltiles], F32, kind="ExternalInput").ap()
    u2c_d = nc.dram_tensor("u2c", [32, blocks], F32, kind="ExternalInput").ap()
    out_d = nc.dram_tensor("out", [32, blocks], F32, kind="ExternalOutput").ap()

    if psum_width == 0:
        psum_width = ucols
    psum_bufs = max(2, (16384 // 4) // psum_width)  # fill all 8 PSUM banks

    with tile.TileContext(nc) as tc, ExitStack() as ctx:
        const_pool = ctx.enter_context(tc.tile_pool(name="const", bufs=1))
        psum_pool = ctx.enter_context(
            tc.tile_pool(name="psum", bufs=psum_bufs, space="PSUM"))
        conv_pool = ctx.enter_context(tc.tile_pool(name="conv", bufs=conv_bufs))

        ut_sb = [const_pool.tile([128, ucols], in_dt, name=f"utsb{k}") for k in range(2)]
        lt_sb = [const_pool.tile([128, m], in_dt, name=f"ltsb{k}") for k in range(2)]
        l2c = const_pool.tile([128, ltiles], F32, name="l2c")
        u2c = const_pool.tile([32, blocks], F32, name="u2c")
        rmin16d = const_pool.tile([128, ucols], F16, name="rmin16d")

        wsrc = const_pool.tile([128, 512], in_dt, name="wsrc")

        from contextlib import nullcontext
        loop_cm = tc.For_i(0, rounds, 1) if rounds > 1 else nullcontext()
        ctx.enter_context(loop_cm)

        if pattern is not None:
            nc.vector.memset(rmin16d[:], 60000.0)
        if warmup_mms:
            # Dummy matmuls during the DMA head keep the PE HAM clock warm
            # (idle >3.4us re-throttles the PE to 1.2 GHz).
            nc.vector.memset(wsrc.bitcast(F32)[:] if in_dt is F32R else wsrc[:], 1.0)
            wpsum = psum_pool.tile([128, psum_width], F32, name="psum", tag="psum")
            for _ in range(warmup_mms):
                nc.tensor.matmul(wpsum[:, :512], wsrc[:, :128], wsrc[:],
                                 start=True, stop=True)

        # Small + U loads first so the main loop can start on L-chunk 0.
        nc.sync.dma_start(l2c[:], l2c_d[:])
        nc.sync.dma_start(u2c[:], u2c_d[:])
        for k in range(2):
            nc.sync.dma_start(ut_sb[k][:], ut_d[k])
        CH = min(1024, m)
        for ci, c0 in enumerate(range(0, m, CH)):
            for k in range(2):
                # dma_split: alternate the big L loads between the two
                # HWDGE engines (SP and ACT) for DMA queue parallelism.
                eng = nc.scalar if (dma_split and (ci + k) % 2) else nc.sync
                eng.dma_start(lt_sb[k][:, c0:c0 + CH], lt_d[k][:, c0:c0 + CH])

        pw = psum_width
        for lt in range(ltiles):
            bias = l2c[:, lt:lt + 1]
            pat = pattern[lt % len(pattern)] if pattern is not None else ""
            for g0 in range(0, ucols, pw):
                psum = psum_pool.tile([128, pw], F32, name="psum", tag="psum")
                for k in range(2):
                    lhsT = lt_sb[k][:, lt * 128:(lt + 1) * 128]
                    for s0 in range(g0, g0 + pw, 512):
                        nc.tensor.matmul(
                            psum[:, s0 - g0:s0 - g0 + 512],
                            lhsT,
                            ut_sb[k][:, s0:s0 + 512],
                            start=(k == 0),
                            stop=(k == 1),
                        )
                if pat == "X":
                    continue  # benchmarking variant: no consumer
                if pat == "A!":  # benchmarking: ACT conv only
                    conva = conv_pool.tile([128, pw], F16, name="conva",
                                           tag="conv")
                    nc.scalar.activation(conva[:], psum[:], AF.Identity,
                                         bias=bias, scale=1.0)
                    continue
                if pat == "D!":  # benchmarking: DVE fused min only
                    nc.vector.scalar_tensor_tensor(
                        rmin16d[:, g0:g0 + pw], psum[:], bias,
                        rmin16d[:, g0:g0 + pw], op0=ALU.add, op1=ALU.min)
                    continue
                # default: ACT 16-bit conversion + DVE min merge. The first
                # L-tile converts straight into rmin16d (no init memset, no
                # merge needed).
                if lt == 0:
                    nc.scalar.activation(rmin16d[:, g0:g0 + pw], psum[:],
                                         AF.Identity, bias=bias, scale=1.0)
                    continue
                conv = conv_pool.tile([128, pw], F16, name="conv", tag="conv")
                aw = pw // act_split
                for a0 in range(0, pw, aw):
                    nc.scalar.activation(conv[:, a0:a0 + aw],
                                         psum[:, a0:a0 + aw], AF.Identity,
                                         bias=bias, scale=1.0)
                nc.vector.tensor_tensor(rmin16d[:, g0:g0 + pw],
                                        rmin16d[:, g0:g0 + pw], conv[:],
                                        op=ALU.min)

        fin = rmin16d
        # Partition reduction: transpose every 32x32 block of fin, min over
        # the free dim within each block -> red[32g + i, b] = min over
        # partitions {32g..32g+31} of column 32b + i. Then two tree levels
        # across the four partition groups (base partitions must be
        # 32-aligned and equal for DVE TT, so realign with tiny DMAs).
        tr = const_pool.tile([128, ucols], F16, name="tr")
        nc.vector.transpose(tr[:], fin[:])
        red = const_pool.tile([128, blocks], F16, name="red")
        nc.vector.tensor_reduce(
            red[:], tr.rearrange("p (b j) -> p b j", j=32),
            axis=mybir.AxisListType.X, op=ALU.min,
        )
        half = const_pool.tile([64, blocks], F16, name="half")
        nc.sync.dma_start(half[:], red[64:128, :])
        nc.vector.tensor_tensor(red[:64, :], red[:64, :], half[:, :], op=ALU.min)
        quart = const_pool.tile([32, blocks], F16, name="quart")
        nc.sync.dma_start(quart[:], red[32:64, :])
        nc.vector.tensor_tensor(red[:32, :], red[:32, :], quart[:, :], op=ALU.min)
        pmin = red[:32, :]
        d2 = const_pool.tile([32, blocks], F32, name="d2")
        nc.vector.tensor_tensor(d2[:], pmin[:], u2c[:], op=ALU.add)
        nc.vector.tensor_scalar_max(d2[:], d2[:], 0.0)
        outt = const_pool.tile([32, blocks], F32, name="outt")
        nc.scalar.activation(outt[:], d2[:], AF.Sqrt)
        nc.sync.dma_start(out_d[:], outt[:])

    nc.compile()
    return nc


# fp16 matmul inputs: same PE rate as fp32r but half the DMA/SBUF footprint;
# measured end-to-end max rel err 1.25e-4 (vs 1.11e-4 with fp32r inputs).
FP16_INPUTS = True


def _get_compiled(ucols: int, m: int, fp16_inputs: bool = FP16_INPUTS):
    key = (ucols, m, fp16_inputs)
    if key not in _COMPILED:
        _COMPILED[key] = _build(ucols, m, fp16_inputs=fp16_inputs)
    return _COMPILED[key]


def _prep_inputs(U: np.ndarray, L: np.ndarray, fp16_inputs: bool = FP16_INPUTS):
    """Host-side sharding / layout prep (transpose, -2 scale, norm rows)."""
    n, d = U.shape
    m = L.shape[0]
    ucols = n // CORES
    mm_dt = np.float16 if fp16_inputs else np.float32
    UTm2 = np.ascontiguousarray((-2.0 * U).T).astype(mm_dt).reshape(2, 128, n)
    LT = np.ascontiguousarray(L.T).astype(mm_dt).reshape(2, 128, m)
    l2 = (L.astype(np.float64) ** 2).sum(1).astype(np.float32)
    u2 = (U.astype(np.float64) ** 2).sum(1).astype(np.float32)
    l2cT = np.ascontiguousarray((l2 - C_SHIFT).reshape(m // 128, 128).T)
    u2c = u2 + C_SHIFT
    in_maps = []
    for i in range(CORES):
        sl = slice(i * ucols, (i + 1) * ucols)
        # Device output layout is [32, ucols//32] with column c = 32*b + i at
        # [i, b]; u2c must match that layout.
        u2c_dev = np.ascontiguousarray(u2c[sl].reshape(ucols // 32, 32).T)
        in_maps.append({
            "ut": np.ascontiguousarray(UTm2[:, :, sl]),
            "lt": LT,
            "l2c": l2cT,
            "u2c": u2c_dev,
        })
    return in_maps


def kernel(**inputs) -> np.ndarray:
    from concourse import bass_utils

    U = np.asarray(inputs["U_z"], dtype=np.float32)
    L = np.asarray(inputs["L_z"], dtype=np.float32)
    n = U.shape[0]
    m = L.shape[0]
    ucols = n // CORES
    nc = _get_compiled(ucols, m, FP16_INPUTS)
    in_maps = _prep_inputs(U, L, FP16_INPUTS)
    res = bass_utils.run_bass_kernel_spmd(nc, in_maps, list(range(CORES)))
    # Per-core output [32, ucols//32] holds column c = 32*b + i at [i, b].
    return np.concatenate(
        [np.ascontiguousarray(r["out"].T).reshape(-1) for r in res.results]
    ).astype(np.float32)


if __name__ == "__main__":
    # Smoke test with random data against a numpy reference.
    rng = np.random.default_rng(0)
    U = rng.standard_normal((N, D), dtype=np.float32)
    L = rng.standard_normal((M, D), dtype=np.float32)
    out = kernel(pred=None, U_z=U, L_z=L)
    d2 = (U * U).sum(1)[:, None] + (L * L).sum(1)[None, :] - 2.0 * U @ L.T
    exp = np.sqrt(np.maximum(d2, 0.0).min(1))
    rel = np.abs(out - exp) / np.maximum(np.abs(exp), 1e-9)
    print("max rel err:", rel.max())



# revision 6
# speedup vs baseline: 24.6303x; 24.6303x over previous
"""Trainium2 Bass kernel for batched nearest-neighbor min-distance.

Problem: for each row u of U_z [16384, 256], compute
    min_{l in L_z [8192, 256]} ||u - l||_2
(`pred` is unused by the reference and ignored.)

Strategy (8 NeuronCores, data-parallel over U rows; L_z replicated):
  d2(u,l) = ||u||^2 + ||l||^2 - 2 u.l, computed with U on PSUM partitions
  and L on the free dim so the min over l is a free-dim reduction that the
  DVE does directly from PSUM (measured ~0.55us per [128,1024] chunk --
  2x the cost-model rate; draining every chunk via DVE paces the whole
  kernel at ~71us, beating any ACT/DVE split, which measures 5x slower
  due to dual-PSUM-reader serialization).

  Matmuls are fp8(e4m3) in DoubleRow perf mode (2 K-subtiles per
  instruction). Per 512-col segment three DoubleRow matmuls accumulate
  into PSUM:
    1. main:      lhsT = fp8(-2 U)^T,            rhs = fp8(L)^T
    2. residual:  lhsT = fp8(-2U - fp8(-2U))^T,  rhs = fp8(L)^T
       (halves the fp8 quantization noise on the dot products)
    3. l2 inject: lhsT = ones[1,2,128],          rhs = (a_l, b_l)
       where a = fp8(l2-256), b = fp8(l2-256-a): adds l2_l - 256 to every
       column exactly to +-0.12 (per-column values can't ride a bias).
  So PSUM = -2 u.l + (l2 - 256).

  Per chunk: DVE tensor_reduce(min) over free -> mins[128, t, c].
  Tail (all [128,16]): d2 = max(min_c(mins) + u2 + 256, 0); out = sqrt.
  End-to-end error vs the exact harness inputs (host simulation):
  max rel err 7.6e-3 (gate 2e-2).
"""

import numpy as np
import ml_dtypes

N, M, D = 16384, 8192, 256
CORES = 8
UCOLS = N // CORES          # 2048 U rows per core
UT = UCOLS // 128           # 16 U-tiles
LCH = 1024                  # L chunk (psum width)
NCH = M // LCH              # 8 L chunks

NP8 = ml_dtypes.float8_e4m3

_COMPILED = {}


def _build(rounds: int = 1, debug: bool = False, pattern: str | None = None,
           residual: bool = True, lch: int = LCH, psum_bufs: int = 4):
    """pattern: None (DVE drains everything) or timing variants:
    'X' no drains; 'B' ACT sigmoid-ish drain on c%4==0 (overlap probe)."""
    from contextlib import ExitStack, nullcontext

    import concourse.bacc as bacc
    import concourse.tile as tile
    from concourse import mybir

    F32 = mybir.dt.float32
    F8 = mybir.dt.float8e4
    AF = mybir.ActivationFunctionType
    ALU = mybir.AluOpType
    DR = mybir.MatmulPerfMode.DoubleRow

    nch = M // lch
    nseg = lch // 512

    nc = bacc.Bacc("TRN2", target_bir_lowering=False, debug=debug)

    uw_d = nc.dram_tensor("uw", [128, 2, UCOLS], F8, kind="ExternalInput").ap()
    uwr_d = nc.dram_tensor("uwr", [128, 2, UCOLS], F8, kind="ExternalInput").ap()
    lt_d = nc.dram_tensor("lt", [128, 2, M], F8, kind="ExternalInput").ap()
    l2p_d = nc.dram_tensor("l2p", [1, 2, M], F8, kind="ExternalInput").ap()
    ones_d = nc.dram_tensor("onesw", [1, 2, 128], F8, kind="ExternalInput").ap()
    u2c_d = nc.dram_tensor("u2c", [128, UT], F32, kind="ExternalInput").ap()
    out_d = nc.dram_tensor("out", [128, UT], F32, kind="ExternalOutput").ap()

    with tile.TileContext(nc) as tc, ExitStack() as ctx:
        const_pool = ctx.enter_context(tc.tile_pool(name="const", bufs=1))
        psum_pool = ctx.enter_context(
            tc.tile_pool(name="psum", bufs=psum_bufs, space="PSUM"))
        junk_pool = ctx.enter_context(tc.tile_pool(name="junk", bufs=2))

        uw = const_pool.tile([128, 2, UCOLS], F8, name="uw")
        uwr = const_pool.tile([128, 2, UCOLS], F8, name="uwr")
        lt = const_pool.tile([128, 2, M], F8, name="lt")
        l2p = const_pool.tile([1, 2, M], F8, name="l2p")
        onesw = const_pool.tile([1, 2, 128], F8, name="onesw")
        u2c = const_pool.tile([128, UT], F32, name="u2c")
        mins = const_pool.tile([128, UT, nch], F32, name="mins")

        loop_cm = tc.For_i(0, rounds, 1) if rounds > 1 else nullcontext()
        ctx.enter_context(loop_cm)

        nc.sync.dma_start(u2c[:], u2c_d[:])
        nc.scalar.dma_start(onesw[:], ones_d[:])
        nc.scalar.dma_start(l2p[:], l2p_d[:])
        nc.sync.dma_start(uw[:], uw_d[:])
        nc.sync.dma_start(lt[:, :, 0:lch], lt_d[:, :, 0:lch])
        nc.sync.dma_start(uwr[:], uwr_d[:])
        for c0 in range(lch, M, lch):
            nc.sync.dma_start(lt[:, :, c0:c0 + lch], lt_d[:, :, c0:c0 + lch])

        for t in range(UT):
            wmain = uw[:, :, t * 128:(t + 1) * 128]
            wres = uwr[:, :, t * 128:(t + 1) * 128]
            for c in range(nch):
                c0 = c * lch
                psum = psum_pool.tile([128, lch], F32, name="ps", tag="ps")
                # weight-major order: one PE weight load per operand set
                for s in range(nseg):
                    nc.tensor.matmul(
                        psum[:, s * 512:(s + 1) * 512], wmain,
                        lt[:, :, c0 + s * 512:c0 + s * 512 + 512],
                        start=True, stop=False, perf_mode=DR)
                if residual:
                    for s in range(nseg):
                        nc.tensor.matmul(
                            psum[:, s * 512:(s + 1) * 512], wres,
                            lt[:, :, c0 + s * 512:c0 + s * 512 + 512],
                            start=False, stop=False, perf_mode=DR)
                for s in range(nseg):
                    nc.tensor.matmul(
                        psum[:, s * 512:(s + 1) * 512], onesw[:],
                        l2p[:, :, c0 + s * 512:c0 + s * 512 + 512],
                        start=False, stop=True, perf_mode=DR)
                if pattern == "X":
                    continue
                if pattern == "B" and c % 4 == 0:
                    junk = junk_pool.tile([128, lch], F32, name="junk",
                                          tag="junk")
                    nc.scalar.activation(junk[:], psum[:], AF.Sigmoid,
                                         scale=-0.01)
                    continue
                nc.vector.tensor_reduce(
                    mins[:, t, c:c + 1], psum[:],
                    axis=mybir.AxisListType.X, op=ALU.min)

        # Tail: d2 = max(min_c mins + u2 + 256, 0); out = sqrt(d2).
        m_all = const_pool.tile([128, UT], F32, name="m_all")
        nc.vector.tensor_reduce(m_all[:], mins[:], axis=mybir.AxisListType.X,
                                op=ALU.min)
        d2m = const_pool.tile([128, UT], F32, name="d2m")
        nc.vector.tensor_tensor(d2m[:], m_all[:], u2c[:], op=ALU.add)
        nc.vector.tensor_scalar_max(d2m[:], d2m[:], 0.0)
        outt = const_pool.tile([128, UT], F32, name="outt")
        nc.scalar.activation(outt[:], d2m[:], AF.Sqrt)
        nc.sync.dma_start(out_d[:], outt[:])

    nc.compile()
    return nc


def _get_compiled(rounds: int = 1):
    if rounds not in _COMPILED:
        _COMPILED[rounds] = _build(rounds)
    return _COMPILED[rounds]


def _prep_inputs(U: np.ndarray, L: np.ndarray):
    """Host-side quantization, layout prep and sharding."""
    n = U.shape[0]
    ucols = n // CORES
    ut = ucols // 128

    Um2 = -2.0 * U
    uwq = Um2.astype(NP8)
    uwr = (Um2 - uwq.astype(np.float32)).astype(NP8)
    Ueff = (uwq.astype(np.float64) + uwr.astype(np.float64)) / -2.0
    u2q = ((Ueff ** 2).sum(1)).astype(np.float32)

    Lq = L.astype(NP8)
    l2q = (Lq.astype(np.float64) ** 2).sum(1)
    r = (l2q - 256.0).astype(np.float32)
    a = r.astype(NP8)
    b = (r - a.astype(np.float32)).astype(NP8)
    l2p = np.ascontiguousarray(np.stack([a, b])[None])   # [1, 2, m] fp8

    def dev_layout(X8):                                # [n,256] -> [128,2,n]
        return np.ascontiguousarray(
            X8.T.reshape(2, 128, -1).transpose(1, 0, 2))

    uw_all = dev_layout(uwq)
    uwr_all = dev_layout(uwr)
    lt = dev_layout(Lq)
    onesw = np.ones((1, 2, 128), dtype=NP8)

    in_maps = []
    for i in range(CORES):
        sl = slice(i * ucols, (i + 1) * ucols)
        u2c = u2q[sl].reshape(ut, 128).T + 256.0       # [128, 16]
        in_maps.append({
            "uw": np.ascontiguousarray(uw_all[:, :, sl]),
            "uwr": np.ascontiguousarray(uwr_all[:, :, sl]),
            "lt": lt,
            "l2p": l2p,
            "onesw": onesw,
            "u2c": np.ascontiguousarray(u2c),
        })
    return in_maps


def kernel(**inputs) -> np.ndarray:
    from concourse import bass_utils

    U = np.asarray(inputs["U_z"], dtype=np.float32)
    L = np.asarray(inputs["L_z"], dtype=np.float32)
    nc = _get_compiled(1)
    in_maps = _prep_inputs(U, L)
    res = bass_utils.run_bass_kernel_spmd(nc, in_maps, list(range(CORES)))
    # out[p, t] holds U row 128*t + p of the core's slice.
    return np.concatenate(
        [np.ascontiguousarray(r["out"].T).reshape(-1) for r in res.results]
    ).astype(np.float32)


if __name__ == "__main__":
    rng = np.random.default_rng(0)
    U = rng.standard_normal((N, D), dtype=np.float32)
    L = rng.standard_normal((M, D), dtype=np.float32)
    out = kernel(pred=None, U_z=U, L_z=L)
    d2 = (U * U).sum(1)[:, None] + (L * L).sum(1)[None, :] - 2.0 * U @ L.T
    exp = np.sqrt(np.maximum(d2, 0.0).min(1))
    rel = np.abs(out - exp) / np.maximum(np.abs(exp), 1e-9)
    print("max rel err:", rel.max(), "mean:", rel.mean())
